# revision 27
# baseline (speedup 1.0000x reference)
"""Trainium2 Bass kernel for nn_CountingAbstraction (sparse_attention).

Math (per batch b):
    cn  = l2_normalize(data[b], axis=-1)
    sim = relu(cn @ cn.T)                       # [N, N]
    counter_pre = sim @ [1 | fixed_v]           # rowsum + sim@posenc, [N, 513]
    counter = softplus(counter_pre @ W_exp + b_exp)
    out = [data | counter] @ W_merge

Device formulation (flash-attention-style fusion, never materializing sim):
    Wt = [1 | fixed_v] @ W_exp                  # [N, M], folds rowsum+Dense
    z.T[m, q] = sum_k Wt[k, m] * relu(cnT_k.T @ cnT_q)[k, q]
    counter.T = softplus(z.T + b_exp)           # per-partition bias
    out[q, :] = rawqT_q.T @ W_merge[:D] + counter.T.T @ W_merge[D:]

The two O(N^2) matmul stages (gram and z) run in fp8e4m3 with the PE's
DoubleRow perf mode (2 contraction subtiles per instruction, 2x MACs/cy;
fp32 PSUM accumulation), cutting PE time for those stages in half vs the
bf16 roofline. Quantization scales (powers of 2, exact in fp: they shift
exponents only):
    cn  * S_CN  (32)   -> fp8   |cn| <= 1, so <= 32 << 240 (e4m3 max)
    sim * S_SB  (128)  -> fp8   sim <= ~1, diagonal hits 1.0 exactly
    Wt  * S_WT  (16)   -> fp8   |Wt| ~< 4 on real inputs; clipped to 240
    pz = S_SB*S_WT * z = S_Z * z   (fp32 PSUM)
The softplus chain descales exactly: the ACT Abs op applies scale=1/S_Z
with the raw bias, the DVE relu half works in the scaled domain with a
pre-scaled bias, and the combine uses scalar_tensor_tensor to fold the
1/S_Z into the add. Measured end-to-end rel err vs the f32 reference:
~0.005 (gate is 2e-2); merge stays bf16.

Host prep (per input set; cached device-side across identical calls):
    cn (f32 l2-normalize then fp8 cast), Wt (f32 matmul of the two weight
    inputs with the fixed posenc then fp8), transposes/casts. The
    O(N^2 (D+M)) work runs on device; host prep is O(N (D+M)).

Sharding: core c handles batch c//2, query-row half c%2 (2048 rows)
against all 4096 keys of that batch. Data-parallel, no collectives.
Key columns (and Wt rows, identically) are rotated per-core so this
core's query rows are always key columns [0:NQ] - the k-sum is
permutation-invariant.

softplus is computed as relu(z+b) [DVE] + ln(1 + exp(-|z+b|)) [ACT],
which is range-safe; Abs/Exp/Ln/Relu all live in the one
natural_log_exp_and_others ACT table so the chain costs no table swaps.
The merge matmuls of chunk ch-1 are emitted between the k-loop and
softplus of chunk ch so the in-order PE stream has work while ACT runs
the softplus chain.
"""

import sys

for _p in ("/opt/trn_rl_repo",):
    if _p not in sys.path:
        sys.path.insert(0, _p)

import numpy as np
import ml_dtypes

import concourse.tile as tile
import concourse.mybir as mybir
from concourse import bacc
from concourse.bass import ts, ds

F32 = mybir.dt.float32
BF16 = mybir.dt.bfloat16
FP8 = mybir.dt.float8e4
AF = mybir.ActivationFunctionType
ALU = mybir.AluOpType
DR = mybir.MatmulPerfMode.DoubleRow
BF = ml_dtypes.bfloat16
E4 = ml_dtypes.float8_e4m3

B, N, D, M = 4, 4096, 512, 512
NCORES = 8
NQ = (B * N) // NCORES  # 2048 query rows per core

S_CN = 32.0
S_SB = 128.0
S_WT = 16.0
S_Z = S_SB * S_WT                    # pz = S_Z * z
RELU_SCALE = S_SB / (S_CN * S_CN)    # psum(gram) -> sb fp8
INV_SZ = 1.0 / S_Z


def _posenc(n, d):
    pos = np.arange(n, dtype=np.float32)[:, None]
    i = np.arange(d // 2, dtype=np.float32)[None, :]
    angle = pos / np.power(10000.0, 2.0 * i / d)
    pe = np.zeros((n, d), dtype=np.float32)
    pe[:, 0::2] = np.sin(angle)
    pe[:, 1::2] = np.cos(angle)
    return pe


def build_nc(nkeys=N, nq=NQ, qch=512, num_cores=NCORES, iters=1):
    """Build the SPMD Bass kernel (identical on every core).

    iters > 1 replicates the whole body (input DMAs included) that many
    times inside one NEFF. The tile pools make copy k+1 reuse copy k's
    buffers behind WAR semaphores, so the copies serialize on device:
    one launch, `iters` full back-to-back executions. test.py uses the
    marginal wall-clock per extra copy as the HW exec time (the fixed
    per-launch tunnel overhead, ~24 ms here, cancels in the difference).
    """
    assert D % 256 == 0 and M % 128 == 0 and nkeys % 512 == 0
    assert nq % qch == 0 and qch % 128 == 0 and qch <= 512
    DP = D // 128       # contraction subtiles over feature dim
    MJ = M // 128       # output-column subtiles
    KB = nkeys // 128   # key blocks
    KP = KB // 2        # key block pairs (fp8 DoubleRow contracts 2 at once)
    NCH = nq // qch     # query chunks

    nc = bacc.Bacc("TRN2", target_bir_lowering=False, debug=False,
                   num_devices=num_cores)
    # wtd/wm/bexp are host pre-tiled to partition-major layout so each
    # loads as one or few wide-line DMAs (4-16KB per partition line)
    # instead of dozens of 512B-line transfers.
    cnd = nc.dram_tensor("cnd", [D, nkeys], FP8, kind="ExternalInput").ap()
    rqd = nc.dram_tensor("rqd", [D, nq], BF16, kind="ExternalInput").ap()
    wtd = nc.dram_tensor("wtd", [128, (nkeys // 128) * M], FP8,
                         kind="ExternalInput").ap()
    wm = nc.dram_tensor("wm", [128, ((D + M) // 128) * M], BF16,
                        kind="ExternalInput").ap()
    # cols 0..MJ-1: raw b_exp (ACT bias); cols MJ..2MJ-1: b_exp * S_Z (DVE)
    bexp = nc.dram_tensor("bexp", [128, 2 * MJ], F32, kind="ExternalInput").ap()
    out = nc.dram_tensor("out", [nq, M], F32, kind="ExternalOutput").ap()

    with tile.TileContext(nc) as tc:
        with (
            tc.tile_pool(name="res", bufs=2) as res,
            tc.tile_pool(name="work", bufs=3) as work,
            tc.tile_pool(name="psg", bufs=3, space="PSUM") as psg,
            tc.tile_pool(name="psz", bufs=MJ, space="PSUM") as psz,
            tc.tile_pool(name="pso", bufs=1, space="PSUM") as pso,
        ):
          for _it in range(iters):
            # ---- residents (double-buffered so iteration k+1's loads run
            # during iteration k's compute) ----------------------------------
            # The gram/z pipeline starts on key chunk 0, so its DMAs go
            # first; rawq/wm/bexp (merge-time consumers, ~30us later) are
            # deferred into the key stream so they don't starve the first
            # grams of key data.
            bexp_sb = res.tile([128, 2 * MJ], F32, tag="bexp", name="bexp_sb")
            wm_sb = res.tile([128, DP + MJ, M], BF16, tag="wm", name="wm_sb")
            wt = res.tile([128, KB, M], FP8, tag="wt", name="wt")
            cnk = res.tile([128, DP, nkeys], FP8, tag="cnk", name="cnk")
            rawq = res.tile([128, DP, nq], BF16, tag="rawq", name="rawq")

            # input staging pieces (keys per piece); the first is small so
            # the very first gram pair starts after ~0.5MB, not ~1MB
            PIECES = (512, 1536, 1024, 1024)
            POFF = [sum(PIECES[:i]) for i in range(len(PIECES))]

            def load_piece(pi):
                off, n = POFF[pi], PIECES[pi]
                for dp in range(DP):
                    nc.sync.dma_start(cnk[:, dp, ds(off, n)],
                                      cnd[ts(dp, 128), ds(off, n)])
                nc.sync.dma_start(wt[:, ds(off // 128, n // 128), :],
                                  wtd[:, ds((off // 128) * M, (n // 128) * M)])
                if pi == 1:
                    nc.sync.dma_start(bexp_sb[:], bexp[:])
                    nc.sync.dma_start(wm_sb[:], wm[:])
                if pi == 2:
                    for dp in range(DP):
                        nc.sync.dma_start(rawq[:, dp, :], rqd[ts(dp, 128), :])

            # ---- fused sim / counter / merge -------------------------------
            # merge(ch-1) is emitted between the k-loop(ch) and softplus(ch):
            # the PE chews merge matmuls (whose cts are long ready) while ACT
            # runs softplus(ch); softplus(ch-1) itself overlapped k-loop(ch).
            # The relu-requant of the gram psum (128 ops of [128,512] f32 ->
            # fp8 per core) would saturate the DVE alone (~0.7us each at the
            # DVE's 1 f32 elem/lane/cycle), so it is split between DVE and
            # ACT (GPSIMD/Pool cannot read PSUM - BIR verifier rejects it;
            # Pool instead owns the SBUF-only ct combines). ACT only takes
            # late pairs: at chunk start its queue still holds the previous
            # chunk's softplus chain (~9us), while DVE only holds the four
            # t1s (~2.6us), so the early pairs' relus land on DVE and z
            # never waits at the boundary.
            def relu_requant(dest, ps, eng):
                if eng == "A":
                    nc.scalar.activation(dest, ps[:], AF.Relu,
                                         scale=RELU_SCALE)
                else:
                    nc.vector.tensor_scalar(dest, ps[:], RELU_SCALE, 0.0,
                                            ALU.mult, ALU.max)

            # per-pair (slot0, slot1) engine assignment within a chunk:
            # 22 DVE / 10 ACT. ACT relus live only in pairs 4..11: earlier
            # ones would queue behind the previous chunk's softplus chain
            # (~6us) and stall their z; later ones would delay the t2s that
            # free the pz banks for the next chunk's z accumulation.
            ACT_RELUS = {(4, 1), (5, 1), (6, 0), (6, 1), (7, 1),
                         (8, 1), (9, 1), (10, 0), (10, 1), (11, 1)}

            def pair_engines(u):
                return ("A" if (u, 0) in ACT_RELUS else "D",
                        "A" if (u, 1) in ACT_RELUS else "D")

            def gram_part(ch, ki, dest, eng="D"):
                # dest: fp8 [128, qch] slice = relu(sim_block) * S_SB
                ps = psg.tile([128, qch], F32, tag="ps", name="ps")
                nc.tensor.matmul(ps[:], cnk[:, 0:2, ts(ki, 128)],
                                 cnk[:, 0:2, ds(ch * qch, qch)],
                                 start=True, stop=False, perf_mode=DR)
                nc.tensor.matmul(ps[:], cnk[:, 2:4, ts(ki, 128)],
                                 cnk[:, 2:4, ds(ch * qch, qch)],
                                 start=False, stop=True, perf_mode=DR)
                relu_requant(dest, ps, eng)

            def z_part(u, sbp, pz):
                for mj in range(MJ):
                    nc.tensor.matmul(pz[mj][:], wt[:, 2 * u:2 * u + 2,
                                                   ts(mj, 128)],
                                     sbp[:], start=(u == 0), stop=(u == KP - 1),
                                     perf_mode=DR)

            class KPipe:
                """Emit z(u-2) after the gram pair u: the PE stream is two
                relu latencies ahead of the pair it is about to consume, so
                neither the boundary t1 backlog on DVE nor a queued relu
                ever stalls a z matmul."""
                def __init__(self, ch, pz, sbtag="sb", sbbufs=4):
                    self.ch, self.pz, self.pending = ch, pz, []
                    self.sbtag, self.sbbufs = sbtag, sbbufs
                def step(self, u):
                    sbp = work.tile([128, 2, qch], FP8, tag=self.sbtag,
                                    bufs=self.sbbufs, name=self.sbtag)
                    e0, e1 = pair_engines(u)
                    gram_part(self.ch, 2 * u, sbp[:, 0, :], e0)
                    gram_part(self.ch, 2 * u + 1, sbp[:, 1, :], e1)
                    self.pending.append((u, sbp))
                    if len(self.pending) > 2:
                        z_part(*self.pending.pop(0), self.pz)
                    return sbp
                def flush(self):
                    for u, sbp in self.pending:
                        z_part(u, sbp, self.pz)
                    self.pending = []

            def emit_merge_group(ch, cts, qs):
                # One 128-row output block: accumulate rawq and counter
                # halves into the pso bank, bounce through SBUF (DMA cannot
                # read PSUM), stream out. Middle chunks weave these groups
                # into the NEXT chunk's k-loop (after its softplus(ch) has
                # drained from ACT), so they never pile up at a boundary.
                po = pso.tile([128, M], F32, tag="po", name="po")
                for dp in range(DP):
                    nc.tensor.matmul(po[:],
                                     rawq[:, dp, ds(ch * qch + qs * 128, 128)],
                                     wm_sb[:, dp, :],
                                     start=(dp == 0), stop=False)
                for mj in range(MJ):
                    nc.tensor.matmul(po[:], cts[mj][:, ts(qs, 128)],
                                     wm_sb[:, DP + mj, :],
                                     start=False, stop=(mj == MJ - 1))
                ob = work.tile([128, M], F32, tag="ob", bufs=2, name="ob")
                nc.vector.tensor_copy(ob[:], po[:])
                nc.sync.dma_start(out[ds(ch * qch + qs * 128, 128), :], ob[:])

            def emit_merge(ch, cts):
                for qs in range(qch // 128):
                    emit_merge_group(ch, cts, qs)

            def emit_t1(pz):
                # DVE half of softplus in the scaled domain (bias = b*S_Z);
                # emitted right after the k-loop so the pz banks free at the
                # chunk boundary, before the ob/merge tail queues behind it.
                t1s = []
                for mj in range(MJ):
                    bs = bexp_sb[:, MJ + mj:MJ + mj + 1]
                    t1 = work.tile([128, qch], F32, tag="t1", bufs=4, name="t1")
                    nc.vector.tensor_scalar(t1[:], pz[mj][:], bs, 0.0,
                                            ALU.add, ALU.max)
                    t1s.append(t1)
                return t1s

            def emit_softplus_rest(pz):
                # counter = softplus(z + b) = relu(z+b) + ln(1 + exp(-|z+b|)),
                # with pz = S_Z * z; Abs descales exactly via the ACT scale
                # operand. The t2s free the pz banks ACT-side right at the
                # boundary; Exp and Ln are batched across mj (all four funcs
                # share the natural_log_exp table, so no reloads). The final
                # combines are deferred to emit_ct (just before the consuming
                # merge).
                t2s, t3s, t4s = [], [], []
                for mj in range(MJ):
                    t2 = work.tile([128, qch], F32, tag="t2", bufs=4, name="t2")
                    nc.scalar.activation(t2[:], pz[mj][:], AF.Abs,
                                         bias=bexp_sb[:, mj:mj + 1],
                                         scale=INV_SZ)
                    t2s.append(t2)
                for mj in range(MJ):
                    t3 = work.tile([128, qch], F32, tag="t3", bufs=4, name="t3")
                    nc.scalar.activation(t3[:], t2s[mj][:], AF.Exp, scale=-1.0)
                    t3s.append(t3)
                for mj in range(MJ):
                    t4 = work.tile([128, qch], F32, tag="t4", bufs=4, name="t4")
                    nc.scalar.activation(t4[:], t3s[mj][:], AF.Ln, bias=1.0)
                    t4s.append(t4)
                return t4s

            def emit_softplus(pz):
                t1s = emit_t1(pz)
                t4s = emit_softplus_rest(pz)
                return list(zip(t1s, t4s))

            def emit_ct(parts):
                # ct = t1/S_Z + t4 on Pool (SBUF-to-SBUF, the only
                # elementwise work Pool may run - it can't read PSUM and
                # rejects the fused scalar_tensor_tensor opcode, so descale
                # and add are two ops; keeps it all off DVE/ACT)
                cts = []
                for t1, t4 in parts:
                    td = work.tile([128, qch], F32, tag="td", bufs=4, name="td")
                    nc.gpsimd.tensor_scalar(td[:], t1[:], INV_SZ, None,
                                            ALU.mult)
                    ct = work.tile([128, qch], BF16, tag="ct", bufs=4, name="ct")
                    nc.gpsimd.tensor_add(ct[:], td[:], t4[:])
                    cts.append(ct)
                return cts

            def alloc_pz():
                return [psz.tile([128, qch], F32, tag="pz", name=f"pz{mj}")
                        for mj in range(MJ)]

            # chunk 0: k-work interleaved with the key/Wt loads, lagging one
            # piece so each piece's DMA latency hides behind the PE work of
            # the previous piece.
            pz0 = alloc_pz()
            pipe0 = KPipe(0, pz0)
            for pi in range(len(PIECES)):
                load_piece(pi)
                if pi >= 1:
                    for u in range(POFF[pi - 1] // 256, POFF[pi] // 256):
                        pipe0.step(u)
            for u in range(POFF[-1] // 256, KP):
                pipe0.step(u)
            pipe0.flush()
            prev = emit_softplus(pz0)

            for ch in range(1, NCH - 1):
                pz = alloc_pz()
                pipe = KPipe(ch, pz)
                cts = None
                for u in range(KP):
                    pipe.step(u)
                    # merge(ch-1) weaves into this chunk's late steps: its
                    # cts depend on softplus(ch-1), which drains from ACT
                    # ~9us into this chunk, and the groups spaced a step
                    # apart never contend for the single pso bank.
                    if u == 8:
                        cts = emit_ct(prev)
                    if u >= KP - 4:
                        emit_merge_group(ch - 1, cts, u - (KP - 4))
                pipe.flush()
                t1s = emit_t1(pz)
                prev = list(zip(t1s, emit_softplus_rest(pz)))

            # Last chunk runs z mj-major: all KP accumulations of one
            # 128-row output block complete 3 z-blocks before the chunk
            # ends, so each block's softplus chain (ACT) pipelines under
            # the next block's z matmuls instead of being exposed as a
            # serial tail after the final k-loop.
            ch = NCH - 1
            pz = alloc_pz()
            sbs = []
            for u in range(KP):
                sbp = work.tile([128, 2, qch], FP8, tag="sbL", bufs=KP + 1,
                                name="sbL")
                e0, e1 = pair_engines(u)
                gram_part(ch, 2 * u, sbp[:, 0, :], e0)
                gram_part(ch, 2 * u + 1, sbp[:, 1, :], e1)
                sbs.append(sbp)
            emit_merge(ch - 1, emit_ct(prev))
            # The last merge accumulates both halves directly in four
            # psum banks (pso + the three now-idle gram banks): the rawq
            # half of block qs is interleaved after z(mj=qs) (no chain
            # dependency), and each counter column-block mj is added to
            # all four banks as soon as its softplus chain and ct combine
            # finish - no merge group ever waits for the full softplus
            # tail, and the old oa/pb/add path disappears.
            pms, cts = [], []
            for mj in range(MJ):
                for u in range(KP):
                    nc.tensor.matmul(pz[mj][:], wt[:, 2 * u:2 * u + 2,
                                                   ts(mj, 128)],
                                     sbs[u][:], start=(u == 0),
                                     stop=(u == KP - 1), perf_mode=DR)
                qs = mj
                pm = pso.tile([128, M], F32, tag="po", name="po") if qs == 0 \
                    else psg.tile([128, M], F32, tag="ps", name="ps")
                for dp in range(DP):
                    nc.tensor.matmul(pm[:],
                                     rawq[:, dp, ds(ch * qch + qs * 128, 128)],
                                     wm_sb[:, dp, :],
                                     start=(dp == 0), stop=False)
                pms.append(pm)
                bs = bexp_sb[:, MJ + mj:MJ + mj + 1]
                t1 = work.tile([128, qch], F32, tag="t1", bufs=4, name="t1")
                nc.vector.tensor_scalar(t1[:], pz[mj][:], bs, 0.0,
                                        ALU.add, ALU.max)
                t2 = work.tile([128, qch], F32, tag="t2", bufs=4, name="t2")
                nc.scalar.activation(t2[:], pz[mj][:], AF.Abs,
                                     bias=bexp_sb[:, mj:mj + 1], scale=INV_SZ)
                t3 = work.tile([128, qch], F32, tag="t3", bufs=4, name="t3")
                nc.scalar.activation(t3[:], t2[:], AF.Exp, scale=-1.0)
                t4 = work.tile([128, qch], F32, tag="t4", bufs=4, name="t4")
                nc.scalar.activation(t4[:], t3[:], AF.Ln, bias=1.0)
                cts.append(emit_ct([(t1, t4)])[0])
            for mj in range(MJ):
                for qs in range(qch // 128):
                    nc.tensor.matmul(pms[qs][:], cts[mj][:, ts(qs, 128)],
                                     wm_sb[:, DP + mj, :],
                                     start=False, stop=(mj == MJ - 1))
            for qs in range(qch // 128):
                ob = work.tile([128, M], F32, tag="ob", bufs=2, name="ob")
                nc.vector.tensor_copy(ob[:], pms[qs][:])
                nc.sync.dma_start(out[ds(ch * qch + qs * 128, 128), :], ob[:])

    nc.compile()
    return nc


def make_in_maps(data, W_exp, b_exp, W_merge, num_cores=NCORES):
    """Host prep: normalize/transpose/fold/quantize inputs into per-core maps."""
    data = np.asarray(data, dtype=np.float32)
    W_exp = np.asarray(W_exp, dtype=np.float32)
    b_exp = np.asarray(b_exp, dtype=np.float32)
    W_merge = np.asarray(W_merge, dtype=np.float32)

    # l2-normalized rows (f32 math, fp8 storage), transposed to [B, D, N]
    sq = np.einsum('bnd,bnd->bn', data, data)
    cn = data * (1.0 / np.sqrt(np.maximum(sq, 1e-12)))[..., None]
    cnT = np.ascontiguousarray(
        (cn * S_CN).transpose(0, 2, 1)).astype(E4)          # |cn|<=1 -> <=32
    dataT = np.ascontiguousarray(data.transpose(0, 2, 1)).astype(BF)

    # Wt = [1 | fixed_v] @ W_exp, f32 on host, fp8 (scaled) on device.
    # Device layouts are partition-major pre-tiled: [128, KB*M] (wt),
    # [128, 8*M] (wm), [128, 2*MJ] (bexp) so they DMA with wide lines.
    def tile_pm(a):   # [C*128, M] -> [128, C*M]
        c = a.shape[0] // 128
        return np.ascontiguousarray(
            a.reshape(c, 128, a.shape[1]).transpose(1, 0, 2).reshape(128, -1))

    wt_full_f = _posenc(N, D) @ W_exp[1:] + W_exp[0:1]      # [N, M]
    wt_full = np.clip(wt_full_f * S_WT, -240.0, 240.0).astype(E4)
    wt_rot = np.roll(wt_full, -NQ, axis=0)
    wt_full = tile_pm(wt_full)
    wt_rot = tile_pm(wt_rot)

    wm_bf = tile_pm(W_merge.astype(BF))
    MJ = M // 128
    bexp_r = np.ascontiguousarray(np.concatenate([
        b_exp.reshape(MJ, 128),
        (b_exp * S_Z).reshape(MJ, 128),
    ], axis=0).astype(np.float32).T)                         # [128, 2*MJ]

    in_maps = []
    for c in range(num_cores):
        b, h = c // 2, c % 2
        # rotate key columns so this core's query rows are always keys
        # [0:NQ]; Wt rows are rotated identically (the k-sum is
        # permutation-invariant).
        if h == 0:
            cnd = cnT[b]
            wtd = wt_full
        else:
            cnd = np.ascontiguousarray(np.roll(cnT[b], -NQ, axis=1))
            wtd = wt_rot
        in_maps.append({
            "cnd": cnd,
            "rqd": np.ascontiguousarray(dataT[b][:, h * NQ:(h + 1) * NQ]),
            "wtd": wtd,
            "wm": wm_bf,
            "bexp": bexp_r,
        })
    return in_maps


_NC_CACHE = {}


def get_nc(iters=1):
    key = ("full", iters)
    if key not in _NC_CACHE:
        _NC_CACHE[key] = build_nc(iters=iters)
    return _NC_CACHE[key]


_EXEC_CACHE = {}


def get_exec(nc):
    """Jitted shard_map executor for `nc`, built once and cached.

    run_bass_kernel_spmd re-traces and re-XLA-compiles on every call
    (fresh closures), costing ~2s per call; caching the jitted fn makes
    warm kernel() calls transfer-bound instead.
    """
    if id(nc) in _EXEC_CACHE:
        return _EXEC_CACHE[id(nc)]

    import jax
    import concourse.mybir as _mybir
    from concourse.bass2jax import (_bass_exec_p, install_neuronx_cc_hook,
                                    partition_id_tensor)
    from jax.sharding import Mesh, PartitionSpec, NamedSharding
    from jax.experimental.shard_map import shard_map

    install_neuronx_cc_hook()
    partition_name = (nc.partition_id_tensor.name
                      if nc.partition_id_tensor else None)
    in_names, out_names, out_avals, zero_outs = [], [], [], []
    for alloc in nc.m.functions[0].allocations:
        if not isinstance(alloc, _mybir.MemoryLocationSet):
            continue
        name = alloc.memorylocations[0].name
        if alloc.kind == "ExternalInput":
            if name != partition_name:
                in_names.append(name)
        elif alloc.kind == "ExternalOutput":
            out_names.append(name)
            shape = tuple(alloc.tensor_shape)
            dtype = _mybir.dt.np(alloc.dtype)
            out_avals.append(jax.core.ShapedArray(shape, dtype))
            zero_outs.append(np.zeros(shape, dtype))
    n_params = len(in_names)
    all_names = in_names + out_names
    if partition_name is not None:
        all_names = all_names + [partition_name]

    def _body(*args):
        operands = list(args)
        if partition_name is not None:
            operands.append(partition_id_tensor())
        outs = _bass_exec_p.bind(
            *operands,
            out_avals=tuple(out_avals),
            in_names=tuple(all_names),
            out_names=tuple(out_names),
            lowering_input_output_aliases=(),
            sim_require_finite=True,
            sim_require_nnan=True,
            nc=nc,
        )
        return tuple(outs)

    devices = jax.devices()[:NCORES]
    mesh = Mesh(np.asarray(devices), ("core",))
    spec = PartitionSpec("core")
    n_outs = len(out_names)
    fn = jax.jit(
        shard_map(_body, mesh=mesh, in_specs=(spec,) * (n_params + n_outs),
                  out_specs=(spec,) * n_outs, check_rep=False),
        keep_unused=True,
    )
    sharding = NamedSharding(mesh, spec)
    zero_dev = [jax.device_put(np.concatenate([z] * NCORES, axis=0), sharding)
                for z in zero_outs]
    state = {
        "fn": fn, "in_names": in_names, "out_names": out_names,
        "out_avals": out_avals, "zero_dev": zero_dev, "sharding": sharding,
        "input_key": None, "dev_in": None,
    }
    _EXEC_CACHE[id(nc)] = state
    return state


def _run_cached(nc, in_maps, fetch=True):
    """Execute nc on cores 0..7; device-caches inputs across identical calls."""
    import jax
    import hashlib
    st = get_exec(nc)
    h = hashlib.blake2b(digest_size=16)
    for m in in_maps:
        for nm in st["in_names"]:
            h.update(np.ascontiguousarray(m[nm]).view(np.uint8).data)
    key = h.hexdigest()
    if st["input_key"] != key:
        per_core = [[np.asarray(m[nm]) for nm in st["in_names"]]
                    for m in in_maps]
        concat_in = [
            np.concatenate([per_core[c][i] for c in range(NCORES)], axis=0)
            for i in range(len(st["in_names"]))
        ]
        st["dev_in"] = [jax.device_put(a, st["sharding"]) for a in concat_in]
        st["input_key"] = key
    outs = st["fn"](*st["dev_in"], *st["zero_dev"])
    jax.block_until_ready(outs)
    if not fetch:
        return outs
    results = []
    for c in range(NCORES):
        results.append({
            name: np.asarray(outs[i]).reshape(
                NCORES, *st["out_avals"][i].shape)[c]
            for i, name in enumerate(st["out_names"])
        })
    return results


def kernel(data, W_exp, b_exp, W_merge):
    nc = get_nc()
    in_maps = make_in_maps(data, W_exp, b_exp, W_merge)
    results = _run_cached(nc, in_maps)
    out = np.empty((B, N, M), dtype=np.float32)
    for c in range(NCORES):
        b, h = c // 2, c % 2
        out[b, h * NQ:(h + 1) * NQ] = results[c]["out"]
    return out
